# revision 15
# baseline (speedup 1.0000x reference)
"""Trainium2 Bass kernel for nn_CalibratedNorm.

The reference module collapses algebraically to a per-(sample, channel)
affine:

    out[b,c,h,w] = x[b,c,h,w] * A[b,c] + S[b,c]

where, with gs/gsh the folded global-BN scale/shift and ms/msh the folded
mean-of-group-BNs scale/shift (all tiny [C] host math):

    alpha[b] = sigmoid( sum_c (alpha_w[c]/HW) * sum_hw x[b,c,:,:] + alpha_b )
    A[b,c]   = gs[c]  + alpha[b] * (ms[c]  - gs[c])
    S[b,c]   = gsh[c] + alpha[b] * (msh[c] - gsh[c])

Strategy: data-parallel over batch, 4 samples per core on 8 cores. The
kernel is HBM-streaming-bound, so x and out travel as bf16 (worst-case
~0.5% of max |out|, well inside the 2e-2 gate): 12.8 MB of HBM traffic
per core. Per core the bf16 x shard ([4,256,3136] = 6.4 MB) stays
resident in SBUF: load once at half-sample (0.8MB) granularity on the
HWDGE SP ring, store once behind the loads (ordering edges keep every
load ahead of every store so alphas resolve ASAP).

The gate dot z_b = sum_c wp[c] * sum_hw x[b,c,:] runs on the otherwise
idle PE: 14 accumulating chunk-matmuls (lhsT = bf16 wp column, rhs =
[128,448] x chunks) collapse both the channel (partition) axis and 7x
of the free axis into one PSUM row [1,448]; ACT finishes it with a
Copy+accum (448 elems) and the sigmoid. This sidesteps the DVE/ACT
accumulator paths, which all run at 1x (~3.1-3.5us per half) - measured;
DVE tensor_reduce likewise only has a 1x uop. DVE is left with just the
fused scale+shift tensor_scalar ops, which hit the 4x bf16 mode
(~1.1us per half), so every engine sits far below the ~31us DMA ring
floor and the ring never starves.
"""

import sys

import numpy as np

for _p in ("/opt/trn_rl_repo",):
    if _p not in sys.path:
        sys.path.insert(0, _p)

import ml_dtypes

import concourse.bacc as bacc
import concourse.bass as bass
import concourse.tile as tile
from concourse import mybir
from concourse.bass_utils import run_bass_kernel_spmd
from concourse.tile import add_dep_helper

EPS = 1e-5
B, C, H, W, G = 32, 256, 56, 56, 32
HW = H * W  # 3136
NCORES = 8
BPC = B // NCORES  # samples per core: 4
HALVES = C // 128  # channel partition-tiles per sample: 2
ROWS = BPC * C  # 1024 rows of the per-core [ROWS, HW] x shard
F32 = mybir.dt.float32
BF16 = mybir.dt.bfloat16
NPAR = 4 * HALVES + 1  # fp32 param cols: tab(4 x HALVES) | ab
CH = 448  # gate-matmul chunk: 7 chunks x 448 = 3136, fits one PSUM bank
NCH = HW // CH


def build_module() -> bass.Bass:
    # Bacc (not raw Bass): its compile() pass splits multi-sem waits into
    # EventSemaphore instructions — TRN2 allows at most 1 wait per
    # compute instruction and walrus codegen hard-errors otherwise.
    nc = bacc.Bacc("TRN2")

    x_in = nc.dram_tensor("x", [BPC * 128, HALVES * HW], BF16, kind="ExternalInput")
    par_in = nc.dram_tensor("par", [128, NPAR], F32, kind="ExternalInput")
    wpb_in = nc.dram_tensor("wpb", [128, HALVES], BF16, kind="ExternalInput")
    y_out = nc.dram_tensor("out", [BPC * 128, HALVES * HW], BF16, kind="ExternalOutput")

    with tile.TileContext(nc) as tc:
        with (
            tc.tile_pool(name="xp", bufs=BPC) as xp,
            tc.tile_pool(name="cs", bufs=1) as cs,
            tc.tile_pool(name="wk", bufs=2) as wk,
            tc.tile_pool(name="ps", bufs=2, space="PSUM") as ps,
        ):
            # Tiny param tables ride the ACT HWDGE ring so the SP ring's
            # first descriptors are bulk x loads.
            par = cs.tile([128, NPAR], F32)
            nc.scalar.dma_start(out=par, in_=par_in[:, :])
            wpb = cs.tile([128, HALVES], BF16)
            nc.scalar.dma_start(out=wpb, in_=wpb_in[:, :])
            tab = par[:, 0 : 4 * HALVES].rearrange("p (f h) -> p f h", f=4)
            ab = par[0:1, NPAR - 1 : NPAR]
            ones_row = cs.tile([1, 128], F32)
            nc.vector.memset(ones_row, 1.0)

            # Host pre-permutes the shard to [b, p, h, w]: each partition's
            # full sample row (both channel halves, 12544 B) is contiguous
            # in DRAM, so a sample moves as ONE [128, 12544B] DMA with 2x
            # bigger descriptors (halves the per-packet overhead).
            xv = x_in[:, :].rearrange("(b p) w -> b p w", p=128)
            yv = y_out[:, :].rearrange("(b p) w -> b p w", p=128)

            # Fully per-sample pipeline: sample b's store chases its own
            # load; no cross-sample barrier anywhere, so the DMA ring
            # never idles between the load phase and the store phase.
            loads = []
            stores = []
            for b in range(BPC):
                xt = xp.tile([128, HALVES, HW], BF16, name=f"xt{b}", tag="xt")
                zrow = ps.tile([1, CH], F32, name=f"zr{b}", tag="zr")
                loads.append(
                    nc.sync.dma_start(
                        out=xt[:, :, :].rearrange("p h w -> p (h w)"),
                        in_=xv[b][:, :],
                    )
                )
                # z_b accumulates on PE: both the channel (partition) axis
                # and 7x of the free axis collapse into one PSUM row.
                for h in range(HALVES):
                    for c in range(NCH):
                        nc.tensor.matmul(
                            zrow[:, :],
                            lhsT=wpb[:, h : h + 1],
                            rhs=xt[:, h, c * CH : (c + 1) * CH],
                            start=(h == 0 and c == 0),
                            stop=(h == HALVES - 1 and c == NCH - 1),
                        )
                # Finish the free axis on ACT (448 elems), then the gate.
                zscr = wk.tile([1, CH], F32, name=f"zs{b}", tag="zs")
                z = wk.tile([1, 1], F32, name=f"z{b}", tag="z")
                nc.scalar.activation(
                    out=zscr, in_=zrow[:, :],
                    func=mybir.ActivationFunctionType.Copy,
                    accum_out=z,
                )
                # alpha = sigmoid(z + alpha_b)
                al = wk.tile([1, 1], F32, name=f"al{b}", tag="al")
                nc.scalar.activation(
                    out=al, in_=z,
                    func=mybir.ActivationFunctionType.Sigmoid,
                    bias=ab, scale=1.0,
                )
                # broadcast alpha to all partitions, move to SBUF
                bc = ps.tile([128, 1], F32, name=f"bc{b}", tag="bc")
                nc.tensor.matmul(
                    bc[:, :], lhsT=ones_row[:, :], rhs=al[:, :],
                    start=True, stop=True,
                )
                ac = wk.tile([128, 1], F32, name=f"ac{b}", tag="ac")
                nc.vector.tensor_copy(out=ac, in_=bc[:, :])

                # A = gs + alpha*dms ; S = gsh + alpha*dmsh   [128,1] each
                A = wk.tile([128, HALVES], F32, name=f"A{b}", tag="A")
                Sh = wk.tile([128, HALVES], F32, name=f"S{b}", tag="S")
                for h in range(HALVES):
                    nc.vector.tensor_scalar(
                        out=A[:, h : h + 1], in0=tab[:, 1, h : h + 1],
                        scalar1=ac, scalar2=tab[:, 0, h : h + 1],
                        op0=mybir.AluOpType.mult, op1=mybir.AluOpType.add,
                    )
                    nc.vector.tensor_scalar(
                        out=Sh[:, h : h + 1], in0=tab[:, 3, h : h + 1],
                        scalar1=ac, scalar2=tab[:, 2, h : h + 1],
                        op0=mybir.AluOpType.mult, op1=mybir.AluOpType.add,
                    )

                # Fused affine on DVE (4x bf16 tensor_scalar), one op per
                # channel half; the sample stores as one [128, 12544B] DMA.
                for h in range(HALVES):
                    nc.vector.tensor_scalar(
                        out=xt[:, h, :], in0=xt[:, h, :],
                        scalar1=A[:, h : h + 1], scalar2=Sh[:, h : h + 1],
                        op0=mybir.AluOpType.mult, op1=mybir.AluOpType.add,
                    )
                stores.append(
                    nc.sync.dma_start(
                        out=yv[b][:, :],
                        in_=xt[:, :, :].rearrange("p h w -> p (h w)"),
                    )
                )

            # Keep every load ahead of every store in the HWDGE ring:
            # ordering-only edges (no sems) from each store to the last
            # load. Without this the scheduler interleaves stores before
            # the last load, which delays the last alphas by ~10us.
            for st in stores:
                add_dep_helper(
                    st.ins, loads[-1].ins, sync=False,
                    reason="loads drain before stores on SP ring",
                )

    nc.compile()
    return nc


_NC_CACHE: list = []


def _get_module() -> bass.Bass:
    if not _NC_CACHE:
        _NC_CACHE.append(build_module())
    return _NC_CACHE[0]


def _prep_in_maps(inputs: dict) -> list[dict]:
    x = np.ascontiguousarray(np.asarray(inputs["x"], dtype=np.float32))
    alpha_w = np.asarray(inputs["alpha_w"], dtype=np.float32)
    alpha_b = np.asarray(inputs["alpha_b"], dtype=np.float32)
    g_w = np.asarray(inputs["g_w"], dtype=np.float32)
    g_b = np.asarray(inputs["g_b"], dtype=np.float32)
    g_rm = np.asarray(inputs["g_rm"], dtype=np.float32)
    g_rv = np.asarray(inputs["g_rv"], dtype=np.float32)
    grp_w = np.asarray(inputs["grp_w"], dtype=np.float32)
    grp_b = np.asarray(inputs["grp_b"], dtype=np.float32)
    grp_rm = np.asarray(inputs["grp_rm"], dtype=np.float32)
    grp_rv = np.asarray(inputs["grp_rv"], dtype=np.float32)

    gs = g_w / np.sqrt(g_rv + EPS)
    gsh = g_b - g_rm * gs
    sg = grp_w / np.sqrt(grp_rv + EPS)  # [G, C]
    ms = sg.mean(axis=0)
    msh = (grp_b - grp_rm * sg).mean(axis=0)
    dms = ms - gs
    dmsh = msh - gsh

    ch = (np.arange(HALVES)[None, :] * 128 + np.arange(128)[:, None])  # [128, HALVES]
    par = np.zeros((128, NPAR), dtype=np.float32)
    par[:, 0 * HALVES : 1 * HALVES] = gs[ch]
    par[:, 1 * HALVES : 2 * HALVES] = dms[ch]
    par[:, 2 * HALVES : 3 * HALVES] = gsh[ch]
    par[:, 3 * HALVES : 4 * HALVES] = dmsh[ch]
    par[0, NPAR - 1] = alpha_b.reshape(-1)[0]
    wpb = (alpha_w[ch] / np.float32(HW)).astype(ml_dtypes.bfloat16)

    # Permute each core's shard to [b, p, h, w] so both channel halves of
    # a partition are DRAM-contiguous (12544 B rows; see build_module).
    xb = np.ascontiguousarray(
        x.reshape(NCORES, BPC, HALVES, 128, HW).transpose(0, 1, 3, 2, 4)
    ).reshape(NCORES, BPC * 128, HALVES * HW).astype(ml_dtypes.bfloat16)
    in_maps = []
    for k in range(NCORES):
        in_maps.append({"x": xb[k], "par": par, "wpb": wpb})
    return in_maps


def _run(inputs: dict, trace: bool = False, trace_cores=None):
    nc = _get_module()
    in_maps = _prep_in_maps(inputs)
    res = run_bass_kernel_spmd(
        nc, in_maps, core_ids=list(range(NCORES)), trace=trace,
        trace_cores=trace_cores,
    )
    outs = [
        np.asarray(r["out"])
        .astype(np.float32)
        .reshape(BPC, 128, HALVES, HW)
        .transpose(0, 2, 1, 3)
        .reshape(BPC, C, H, W)
        for r in res.results
    ]
    full = np.concatenate(outs, axis=0)
    return full, res


def kernel(**inputs) -> np.ndarray:
    out, _ = _run(inputs, trace=False)
    return out


# revision 18
# speedup vs baseline: 1.1262x; 1.1262x over previous
"""Trainium2 Bass kernel for nn_CalibratedNorm.

The reference module collapses algebraically to a per-(sample, channel)
affine:

    out[b,c,h,w] = x[b,c,h,w] * A[b,c] + S[b,c]

where, with gs/gsh the folded global-BN scale/shift and ms/msh the folded
mean-of-group-BNs scale/shift (all tiny [C] host math):

    alpha[b] = sigmoid( sum_c (alpha_w[c]/HW) * sum_hw x[b,c,:,:] + alpha_b )
    A[b,c]   = gs[c]  + alpha[b] * (ms[c]  - gs[c])
    S[b,c]   = gsh[c] + alpha[b] * (msh[c] - gsh[c])

Strategy: data-parallel over batch, 4 samples per core on 8 cores. The
kernel is HBM-streaming-bound, so x and out travel as bf16 (worst-case
~0.5% of max |out|, well inside the 2e-2 gate): 12.8 MB of HBM traffic
per core. Per core the bf16 x shard ([4,256,3136] = 6.4 MB) stays
resident in SBUF: load once at half-sample (0.8MB) granularity on the
HWDGE SP ring, store once behind the loads (ordering edges keep every
load ahead of every store so alphas resolve ASAP).

The gate dot z_b = sum_c wp[c] * sum_hw x[b,c,:] runs on the otherwise
idle PE: 14 accumulating chunk-matmuls (lhsT = bf16 wp column, rhs =
[128,448] x chunks) collapse both the channel (partition) axis and 7x
of the free axis into one PSUM row [1,448]; ACT finishes it with a
Copy+accum (448 elems) and the sigmoid. This sidesteps the DVE/ACT
accumulator paths, which all run at 1x (~3.1-3.5us per half) - measured;
DVE tensor_reduce likewise only has a 1x uop. DVE is left with just the
fused scale+shift tensor_scalar ops, which hit the 4x bf16 mode
(~1.1us per half), so every engine sits far below the ~31us DMA ring
floor and the ring never starves.
"""

import sys

import numpy as np

for _p in ("/opt/trn_rl_repo",):
    if _p not in sys.path:
        sys.path.insert(0, _p)

import ml_dtypes

import concourse.bacc as bacc
import concourse.bass as bass
import concourse.tile as tile
from concourse import mybir
from concourse.bass_utils import run_bass_kernel_spmd
from concourse.tile import add_dep_helper

EPS = 1e-5
B, C, H, W, G = 32, 256, 56, 56, 32
HW = H * W  # 3136
NCORES = 8
BPC = B // NCORES  # samples per core: 4
HALVES = C // 128  # channel partition-tiles per sample: 2
ROWS = BPC * C  # 1024 rows of the per-core [ROWS, HW] x shard
F32 = mybir.dt.float32
BF16 = mybir.dt.bfloat16
NPAR = 4 * HALVES + 1  # fp32 param cols: tab(4 x HALVES) | ab
CH = 448  # gate-matmul chunk: 7 chunks x 448 = 3136, fits one PSUM bank
NCH = HW // CH


def build_module() -> bass.Bass:
    # Bacc (not raw Bass): its compile() pass splits multi-sem waits into
    # EventSemaphore instructions — TRN2 allows at most 1 wait per
    # compute instruction and walrus codegen hard-errors otherwise.
    nc = bacc.Bacc("TRN2")

    x_in = nc.dram_tensor("x", [BPC * 128, HALVES * HW], BF16, kind="ExternalInput")
    par_in = nc.dram_tensor("par", [128, NPAR], F32, kind="ExternalInput")
    wpb_in = nc.dram_tensor("wpb", [128, HALVES], BF16, kind="ExternalInput")
    y_out = nc.dram_tensor("out", [BPC * 128, HALVES * HW], BF16, kind="ExternalOutput")

    with tile.TileContext(nc) as tc:
        with (
            tc.tile_pool(name="xp", bufs=BPC) as xp,
            tc.tile_pool(name="cs", bufs=1) as cs,
            tc.tile_pool(name="wk", bufs=2) as wk,
            tc.tile_pool(name="ps", bufs=2, space="PSUM") as ps,
        ):
            # Tiny param tables go FIRST on the same sync ring as the bulk
            # x traffic: in-queue FIFO drains their 256 tiny descriptors in
            # ~1.5us. On any OTHER queue they round-robin one descriptor
            # per bulk-packet turn and the gate weights arrive ~12us late,
            # stalling every alpha and starving the ring of stores.
            par = cs.tile([128, NPAR], F32)
            nc.sync.dma_start(out=par, in_=par_in[:, :])
            wpb = cs.tile([128, HALVES], BF16)
            nc.sync.dma_start(out=wpb, in_=wpb_in[:, :])
            tab = par[:, 0 : 4 * HALVES].rearrange("p (f h) -> p f h", f=4)
            ab = par[0:1, NPAR - 1 : NPAR]
            ones_row = cs.tile([1, 128], F32)
            nc.vector.memset(ones_row, 1.0)

            # Host pre-permutes the shard to [b, p, h, w]: each partition's
            # full sample row (both channel halves, 12544 B) is contiguous
            # in DRAM, so a sample moves as ONE [128, 12544B] DMA with 2x
            # bigger descriptors (halves the per-packet overhead).
            xv = x_in[:, :].rearrange("(b p) w -> b p w", p=128)
            yv = y_out[:, :].rearrange("(b p) w -> b p w", p=128)

            # Fully per-sample pipeline: sample b's store chases its own
            # load; no cross-sample barrier anywhere, so the DMA ring
            # never idles between the load phase and the store phase.
            loads = []
            stores = []
            for b in range(BPC):
                xt = xp.tile([128, HALVES, HW], BF16, name=f"xt{b}", tag="xt")
                zrow = ps.tile([1, CH], F32, name=f"zr{b}", tag="zr")
                loads.append(
                    nc.sync.dma_start(
                        out=xt[:, :, :].rearrange("p h w -> p (h w)"),
                        in_=xv[b][:, :],
                    )
                )
                # z_b accumulates on PE: both the channel (partition) axis
                # and 7x of the free axis collapse into one PSUM row.
                for h in range(HALVES):
                    for c in range(NCH):
                        nc.tensor.matmul(
                            zrow[:, :],
                            lhsT=wpb[:, h : h + 1],
                            rhs=xt[:, h, c * CH : (c + 1) * CH],
                            start=(h == 0 and c == 0),
                            stop=(h == HALVES - 1 and c == NCH - 1),
                        )
                # Finish the free axis on ACT (448 elems), then the gate.
                zscr = wk.tile([1, CH], F32, name=f"zs{b}", tag="zs")
                z = wk.tile([1, 1], F32, name=f"z{b}", tag="z")
                nc.scalar.activation(
                    out=zscr, in_=zrow[:, :],
                    func=mybir.ActivationFunctionType.Copy,
                    accum_out=z,
                )
                # alpha = sigmoid(z + alpha_b)
                al = wk.tile([1, 1], F32, name=f"al{b}", tag="al")
                nc.scalar.activation(
                    out=al, in_=z,
                    func=mybir.ActivationFunctionType.Sigmoid,
                    bias=ab, scale=1.0,
                )
                # broadcast alpha to all partitions, move to SBUF
                bc = ps.tile([128, 1], F32, name=f"bc{b}", tag="bc")
                nc.tensor.matmul(
                    bc[:, :], lhsT=ones_row[:, :], rhs=al[:, :],
                    start=True, stop=True,
                )
                ac = wk.tile([128, 1], F32, name=f"ac{b}", tag="ac")
                nc.vector.tensor_copy(out=ac, in_=bc[:, :])

                # A = gs + alpha*dms ; S = gsh + alpha*dmsh   [128,1] each
                A = wk.tile([128, HALVES], F32, name=f"A{b}", tag="A")
                Sh = wk.tile([128, HALVES], F32, name=f"S{b}", tag="S")
                for h in range(HALVES):
                    nc.vector.tensor_scalar(
                        out=A[:, h : h + 1], in0=tab[:, 1, h : h + 1],
                        scalar1=ac, scalar2=tab[:, 0, h : h + 1],
                        op0=mybir.AluOpType.mult, op1=mybir.AluOpType.add,
                    )
                    nc.vector.tensor_scalar(
                        out=Sh[:, h : h + 1], in0=tab[:, 3, h : h + 1],
                        scalar1=ac, scalar2=tab[:, 2, h : h + 1],
                        op0=mybir.AluOpType.mult, op1=mybir.AluOpType.add,
                    )

                # Fused affine on DVE (4x bf16 tensor_scalar), one op per
                # channel half; the sample stores as one [128, 12544B] DMA.
                for h in range(HALVES):
                    nc.vector.tensor_scalar(
                        out=xt[:, h, :], in0=xt[:, h, :],
                        scalar1=A[:, h : h + 1], scalar2=Sh[:, h : h + 1],
                        op0=mybir.AluOpType.mult, op1=mybir.AluOpType.add,
                    )
                stores.append(
                    nc.sync.dma_start(
                        out=yv[b][:, :],
                        in_=xt[:, :, :].rearrange("p h w -> p (h w)"),
                    )
                )

            # Keep every load ahead of every store in the HWDGE ring:
            # ordering-only edges (no sems) from each store to the last
            # load. Without this the scheduler interleaves stores before
            # the last load, which delays the last alphas by ~10us.
            for st in stores:
                add_dep_helper(
                    st.ins, loads[-1].ins, sync=False,
                    reason="loads drain before stores on SP ring",
                )

    nc.compile()
    return nc


_NC_CACHE: list = []


def _get_module() -> bass.Bass:
    if not _NC_CACHE:
        _NC_CACHE.append(build_module())
    return _NC_CACHE[0]


def _prep_in_maps(inputs: dict) -> list[dict]:
    x = np.ascontiguousarray(np.asarray(inputs["x"], dtype=np.float32))
    alpha_w = np.asarray(inputs["alpha_w"], dtype=np.float32)
    alpha_b = np.asarray(inputs["alpha_b"], dtype=np.float32)
    g_w = np.asarray(inputs["g_w"], dtype=np.float32)
    g_b = np.asarray(inputs["g_b"], dtype=np.float32)
    g_rm = np.asarray(inputs["g_rm"], dtype=np.float32)
    g_rv = np.asarray(inputs["g_rv"], dtype=np.float32)
    grp_w = np.asarray(inputs["grp_w"], dtype=np.float32)
    grp_b = np.asarray(inputs["grp_b"], dtype=np.float32)
    grp_rm = np.asarray(inputs["grp_rm"], dtype=np.float32)
    grp_rv = np.asarray(inputs["grp_rv"], dtype=np.float32)

    gs = g_w / np.sqrt(g_rv + EPS)
    gsh = g_b - g_rm * gs
    sg = grp_w / np.sqrt(grp_rv + EPS)  # [G, C]
    ms = sg.mean(axis=0)
    msh = (grp_b - grp_rm * sg).mean(axis=0)
    dms = ms - gs
    dmsh = msh - gsh

    ch = (np.arange(HALVES)[None, :] * 128 + np.arange(128)[:, None])  # [128, HALVES]
    par = np.zeros((128, NPAR), dtype=np.float32)
    par[:, 0 * HALVES : 1 * HALVES] = gs[ch]
    par[:, 1 * HALVES : 2 * HALVES] = dms[ch]
    par[:, 2 * HALVES : 3 * HALVES] = gsh[ch]
    par[:, 3 * HALVES : 4 * HALVES] = dmsh[ch]
    par[0, NPAR - 1] = alpha_b.reshape(-1)[0]
    wpb = (alpha_w[ch] / np.float32(HW)).astype(ml_dtypes.bfloat16)

    # Permute each core's shard to [b, p, h, w] so both channel halves of
    # a partition are DRAM-contiguous (12544 B rows; see build_module).
    xb = np.ascontiguousarray(
        x.reshape(NCORES, BPC, HALVES, 128, HW).transpose(0, 1, 3, 2, 4)
    ).reshape(NCORES, BPC * 128, HALVES * HW).astype(ml_dtypes.bfloat16)
    in_maps = []
    for k in range(NCORES):
        in_maps.append({"x": xb[k], "par": par, "wpb": wpb})
    return in_maps


def _unpermute_core(y: np.ndarray) -> np.ndarray:
    """Inverse of the host-side [b, p, h, w] shard permutation."""
    return (
        y.astype(np.float32)
        .reshape(BPC, 128, HALVES, HW)
        .transpose(0, 2, 1, 3)
        .reshape(BPC, C, H, W)
    )


def _run(inputs: dict, trace: bool = False, trace_cores=None):
    nc = _get_module()
    in_maps = _prep_in_maps(inputs)
    res = run_bass_kernel_spmd(
        nc, in_maps, core_ids=list(range(NCORES)), trace=trace,
        trace_cores=trace_cores,
    )
    outs = [_unpermute_core(np.asarray(r["out"])) for r in res.results]
    full = np.concatenate(outs, axis=0)
    return full, res


def kernel(**inputs) -> np.ndarray:
    out, _ = _run(inputs, trace=False)
    return out


# revision 20
# speedup vs baseline: 1.1506x; 1.0217x over previous
"""Trainium2 Bass kernel for nn_CalibratedNorm.

The reference module collapses algebraically to a per-(sample, channel)
affine:

    out[b,c,h,w] = x[b,c,h,w] * A[b,c] + S[b,c]

where, with gs/gsh the folded global-BN scale/shift and ms/msh the folded
mean-of-group-BNs scale/shift (all tiny [C] host math):

    alpha[b] = sigmoid( sum_c (alpha_w[c]/HW) * sum_hw x[b,c,:,:] + alpha_b )
    A[b,c]   = gs[c]  + alpha[b] * (ms[c]  - gs[c])
    S[b,c]   = gsh[c] + alpha[b] * (msh[c] - gsh[c])

Strategy: data-parallel over batch, 4 samples per core on 8 cores. The
kernel is HBM-streaming-bound, so x and out travel as bf16 (worst-case
~0.5% of max |out|, well inside the 2e-2 gate): 12.8 MB of HBM traffic
per core. Per core the bf16 x shard ([4,256,3136] = 6.4 MB) stays
resident in SBUF: load once at half-sample (0.8MB) granularity on the
HWDGE SP ring, store once behind the loads (ordering edges keep every
load ahead of every store so alphas resolve ASAP).

The gate dot z_b = sum_c wp[c] * sum_hw x[b,c,:] runs on the otherwise
idle PE: 14 accumulating chunk-matmuls (lhsT = bf16 wp column, rhs =
[128,448] x chunks) collapse both the channel (partition) axis and 7x
of the free axis into one PSUM row [1,448]; ACT finishes it with a
Copy+accum (448 elems) and the sigmoid. This sidesteps the DVE/ACT
accumulator paths, which all run at 1x (~3.1-3.5us per half) - measured;
DVE tensor_reduce likewise only has a 1x uop. DVE is left with just the
fused scale+shift tensor_scalar ops, which hit the 4x bf16 mode
(~1.1us per half), so every engine sits far below the ~31us DMA ring
floor and the ring never starves.
"""

import sys

import numpy as np

for _p in ("/opt/trn_rl_repo",):
    if _p not in sys.path:
        sys.path.insert(0, _p)

import ml_dtypes

import concourse.bacc as bacc
import concourse.bass as bass
import concourse.tile as tile
from concourse import mybir
from concourse.bass_utils import run_bass_kernel_spmd
from concourse.tile import add_dep_helper

EPS = 1e-5
B, C, H, W, G = 32, 256, 56, 56, 32
HW = H * W  # 3136
NCORES = 8
BPC = B // NCORES  # samples per core: 4
HALVES = C // 128  # channel partition-tiles per sample: 2
ROWS = BPC * C  # 1024 rows of the per-core [ROWS, HW] x shard
F32 = mybir.dt.float32
BF16 = mybir.dt.bfloat16
NPAR = 4 * HALVES + 1  # fp32 param cols: tab(4 x HALVES) | ab
CH = 448  # gate-matmul chunk: 7 chunks x 448 = 3136, fits one PSUM bank
NCH = HW // CH


def build_module() -> bass.Bass:
    # Bacc (not raw Bass): its compile() pass splits multi-sem waits into
    # EventSemaphore instructions — TRN2 allows at most 1 wait per
    # compute instruction and walrus codegen hard-errors otherwise.
    nc = bacc.Bacc("TRN2")

    x_in = nc.dram_tensor("x", [BPC * 128, HALVES * HW], BF16, kind="ExternalInput")
    par_in = nc.dram_tensor("par", [128, NPAR], F32, kind="ExternalInput")
    wpb_in = nc.dram_tensor("wpb", [128, HALVES], BF16, kind="ExternalInput")
    y_out = nc.dram_tensor("out", [BPC * 128, HALVES * HW], BF16, kind="ExternalOutput")

    with tile.TileContext(nc) as tc:
        with (
            tc.tile_pool(name="xp", bufs=BPC) as xp,
            tc.tile_pool(name="cs", bufs=1) as cs,
            tc.tile_pool(name="wk", bufs=2) as wk,
            tc.tile_pool(name="ps", bufs=2, space="PSUM") as ps,
        ):
            # Tiny param tables go FIRST on the same sync ring as the bulk
            # x traffic: in-queue FIFO drains their 256 tiny descriptors in
            # ~1.5us. On any OTHER queue they round-robin one descriptor
            # per bulk-packet turn and the gate weights arrive ~12us late,
            # stalling every alpha and starving the ring of stores.
            par = cs.tile([128, NPAR], F32)
            nc.sync.dma_start(out=par, in_=par_in[:, :])
            wpb = cs.tile([128, HALVES], BF16)
            nc.sync.dma_start(out=wpb, in_=wpb_in[:, :])
            tab = par[:, 0 : 4 * HALVES].rearrange("p (f h) -> p f h", f=4)
            ab = par[0:1, NPAR - 1 : NPAR]
            ones_row = cs.tile([1, 128], F32)
            nc.vector.memset(ones_row, 1.0)

            # Host pre-permutes the shard to [b, p, h, w]: each partition's
            # full sample row (both channel halves, 12544 B) is contiguous
            # in DRAM, so a sample moves as ONE [128, 12544B] DMA with 2x
            # bigger descriptors (halves the per-packet overhead).
            xv = x_in[:, :].rearrange("(b p) w -> b p w", p=128)
            yv = y_out[:, :].rearrange("(b p) w -> b p w", p=128)

            # Fully per-sample pipeline: sample b's store chases its own
            # load; no cross-sample barrier anywhere, so the DMA ring
            # never idles between the load phase and the store phase.
            loads = []
            stores = []
            for b in range(BPC):
                xt = xp.tile([128, HALVES, HW], BF16, name=f"xt{b}", tag="xt")
                zrow = ps.tile([1, CH], F32, name=f"zr{b}", tag="zr")
                # Half-sample (0.8MB) load granularity: half h's gate
                # matmuls run while half h^1 is still streaming in.
                # z_b accumulates on PE: both the channel (partition) axis
                # and 7x of the free axis collapse into one PSUM row.
                for h in range(HALVES):
                    loads.append(
                        nc.sync.dma_start(
                            out=xt[:, h, :], in_=xv[b][:, h * HW : (h + 1) * HW]
                        )
                    )
                    for c in range(NCH):
                        nc.tensor.matmul(
                            zrow[:, :],
                            lhsT=wpb[:, h : h + 1],
                            rhs=xt[:, h, c * CH : (c + 1) * CH],
                            start=(h == 0 and c == 0),
                            stop=(h == HALVES - 1 and c == NCH - 1),
                        )
                # Finish the free axis on ACT (448 elems), then the gate.
                zscr = wk.tile([1, CH], F32, name=f"zs{b}", tag="zs")
                z = wk.tile([1, 1], F32, name=f"z{b}", tag="z")
                nc.scalar.activation(
                    out=zscr, in_=zrow[:, :],
                    func=mybir.ActivationFunctionType.Copy,
                    accum_out=z,
                )
                # alpha = sigmoid(z + alpha_b)
                al = wk.tile([1, 1], F32, name=f"al{b}", tag="al")
                nc.scalar.activation(
                    out=al, in_=z,
                    func=mybir.ActivationFunctionType.Sigmoid,
                    bias=ab, scale=1.0,
                )
                # broadcast alpha to all partitions, move to SBUF
                bc = ps.tile([128, 1], F32, name=f"bc{b}", tag="bc")
                nc.tensor.matmul(
                    bc[:, :], lhsT=ones_row[:, :], rhs=al[:, :],
                    start=True, stop=True,
                )
                ac = wk.tile([128, 1], F32, name=f"ac{b}", tag="ac")
                nc.vector.tensor_copy(out=ac, in_=bc[:, :])

                # A = gs + alpha*dms ; S = gsh + alpha*dmsh   [128,1] each
                A = wk.tile([128, HALVES], F32, name=f"A{b}", tag="A")
                Sh = wk.tile([128, HALVES], F32, name=f"S{b}", tag="S")
                for h in range(HALVES):
                    nc.vector.tensor_scalar(
                        out=A[:, h : h + 1], in0=tab[:, 1, h : h + 1],
                        scalar1=ac, scalar2=tab[:, 0, h : h + 1],
                        op0=mybir.AluOpType.mult, op1=mybir.AluOpType.add,
                    )
                    nc.vector.tensor_scalar(
                        out=Sh[:, h : h + 1], in0=tab[:, 3, h : h + 1],
                        scalar1=ac, scalar2=tab[:, 2, h : h + 1],
                        op0=mybir.AluOpType.mult, op1=mybir.AluOpType.add,
                    )

                # Fused affine on DVE (4x bf16 tensor_scalar); store each
                # half as soon as its own affine is done.
                for h in range(HALVES):
                    nc.vector.tensor_scalar(
                        out=xt[:, h, :], in0=xt[:, h, :],
                        scalar1=A[:, h : h + 1], scalar2=Sh[:, h : h + 1],
                        op0=mybir.AluOpType.mult, op1=mybir.AluOpType.add,
                    )
                    stores.append(
                        nc.sync.dma_start(
                            out=yv[b][:, h * HW : (h + 1) * HW], in_=xt[:, h, :]
                        )
                    )

            # Keep every load ahead of every store in the HWDGE ring:
            # ordering-only edges (no sems) from each store to the last
            # load. Without this the scheduler interleaves stores before
            # the last load, which delays the last alphas by ~10us.
            for st in stores:
                add_dep_helper(
                    st.ins, loads[-1].ins, sync=False,
                    reason="loads drain before stores on SP ring",
                )

    nc.compile()
    return nc


_NC_CACHE: list = []


def _get_module() -> bass.Bass:
    if not _NC_CACHE:
        _NC_CACHE.append(build_module())
    return _NC_CACHE[0]


def _prep_in_maps(inputs: dict) -> list[dict]:
    x = np.ascontiguousarray(np.asarray(inputs["x"], dtype=np.float32))
    alpha_w = np.asarray(inputs["alpha_w"], dtype=np.float32)
    alpha_b = np.asarray(inputs["alpha_b"], dtype=np.float32)
    g_w = np.asarray(inputs["g_w"], dtype=np.float32)
    g_b = np.asarray(inputs["g_b"], dtype=np.float32)
    g_rm = np.asarray(inputs["g_rm"], dtype=np.float32)
    g_rv = np.asarray(inputs["g_rv"], dtype=np.float32)
    grp_w = np.asarray(inputs["grp_w"], dtype=np.float32)
    grp_b = np.asarray(inputs["grp_b"], dtype=np.float32)
    grp_rm = np.asarray(inputs["grp_rm"], dtype=np.float32)
    grp_rv = np.asarray(inputs["grp_rv"], dtype=np.float32)

    gs = g_w / np.sqrt(g_rv + EPS)
    gsh = g_b - g_rm * gs
    sg = grp_w / np.sqrt(grp_rv + EPS)  # [G, C]
    ms = sg.mean(axis=0)
    msh = (grp_b - grp_rm * sg).mean(axis=0)
    dms = ms - gs
    dmsh = msh - gsh

    ch = (np.arange(HALVES)[None, :] * 128 + np.arange(128)[:, None])  # [128, HALVES]
    par = np.zeros((128, NPAR), dtype=np.float32)
    par[:, 0 * HALVES : 1 * HALVES] = gs[ch]
    par[:, 1 * HALVES : 2 * HALVES] = dms[ch]
    par[:, 2 * HALVES : 3 * HALVES] = gsh[ch]
    par[:, 3 * HALVES : 4 * HALVES] = dmsh[ch]
    par[0, NPAR - 1] = alpha_b.reshape(-1)[0]
    wpb = (alpha_w[ch] / np.float32(HW)).astype(ml_dtypes.bfloat16)

    # Permute each core's shard to [b, p, h, w] so both channel halves of
    # a partition are DRAM-contiguous (12544 B rows; see build_module).
    xb = np.ascontiguousarray(
        x.reshape(NCORES, BPC, HALVES, 128, HW).transpose(0, 1, 3, 2, 4)
    ).reshape(NCORES, BPC * 128, HALVES * HW).astype(ml_dtypes.bfloat16)
    in_maps = []
    for k in range(NCORES):
        in_maps.append({"x": xb[k], "par": par, "wpb": wpb})
    return in_maps


def _unpermute_core(y: np.ndarray) -> np.ndarray:
    """Inverse of the host-side [b, p, h, w] shard permutation."""
    return (
        y.astype(np.float32)
        .reshape(BPC, 128, HALVES, HW)
        .transpose(0, 2, 1, 3)
        .reshape(BPC, C, H, W)
    )


def _run(inputs: dict, trace: bool = False, trace_cores=None):
    nc = _get_module()
    in_maps = _prep_in_maps(inputs)
    res = run_bass_kernel_spmd(
        nc, in_maps, core_ids=list(range(NCORES)), trace=trace,
        trace_cores=trace_cores,
    )
    outs = [_unpermute_core(np.asarray(r["out"])) for r in res.results]
    full = np.concatenate(outs, axis=0)
    return full, res


def kernel(**inputs) -> np.ndarray:
    out, _ = _run(inputs, trace=False)
    return out
